# revision 10
# baseline (speedup 1.0000x reference)
"""CGC layer (MoE routing) kernel for 8 Trainium2 NeuronCores.

Strategy: data-parallel over the batch (8192 / 8 = 1024 rows per core,
params replicated, no collectives).

Per core, for each of 16 experts (4 shared + 12 task-specific) and each
128-row batch tile:
  z[b,h] (PSUM, fp32) = bias ones-matmul (K=1) + sum over 8 K-chunks of
                        bf16 matmuls with the x-transpose chunk stationary
  r = Relu(z)                            (ScalarE, PSUM -> SBUF)
  acc[t] += gate[t,b,e] * r              (VectorE scalar_tensor_tensor,
                                          per-partition gate scalar)
Gates: one bf16 GEMM (K=1024, N=24) per batch tile + softmax per task
group of 8 (Exp on ScalarE, sum/reciprocal/scale on VectorE).

Inputs are sharded + transposed + cast to bf16 on the host; output is
gathered on the host. All shapes hardcoded for the problem instance.
"""

import numpy as np
import ml_dtypes

import concourse.bass as bass
import concourse.mybir as mybir
import concourse.tile as tile
from concourse import bacc
from concourse.bass_utils import run_bass_kernel_spmd

D = 1024          # d_model
H = 1024          # expert_dim
T = 3             # tasks
NSHARED = 4
NSPEC = 4
NE = NSHARED + T * NSPEC    # 16 experts total (shared first)
NG = NSPEC + NSHARED        # 8 gate candidates per task
B = 8192
N_CORES = 8
BL = B // N_CORES           # 1024 rows per core
P = 128                     # partitions
KC = D // P                 # 8 K-chunks
NT = BL // P                # 8 batch tiles per core
NH = H // 512               # 2 PSUM half-tiles

F32 = mybir.dt.float32
BF16 = mybir.dt.bfloat16
ACT = mybir.ActivationFunctionType
ALU = mybir.AluOpType
AXIS = mybir.AxisListType

BF16_NP = ml_dtypes.bfloat16


def _consumers(e):
    """Expert index -> list of (task, gate column in the 24-wide layout)."""
    if e < NSHARED:
        return [(t, t * NG + NSPEC + e) for t in range(T)]
    t, j = divmod(e - NSHARED, NSPEC)
    return [(t, t * NG + j)]


BIAS_MODE = "act"   # "mm": K=1 ones-matmul into PSUM; "act": ScalarE prefill


def _build_nc(repeat=1, bias_mode=None):
    """repeat>1 re-runs the whole compute body (timing builds only)."""
    if bias_mode is None:
        bias_mode = BIAS_MODE
    nc = bacc.Bacc(None, target_bir_lowering=False)

    xT_d = nc.dram_tensor("xT", (D, BL), BF16, kind="ExternalInput")
    w_d = nc.dram_tensor("W", (NE, D, H), BF16, kind="ExternalInput")
    if bias_mode == "act":
        b_d = nc.dram_tensor("bias", (NE, P, H), BF16, kind="ExternalInput")
    else:
        b_d = nc.dram_tensor("bias", (NE, H), BF16, kind="ExternalInput")
    wg_d = nc.dram_tensor("Wg", (D, T * NG), BF16, kind="ExternalInput")
    out_d = nc.dram_tensor("out", (T, BL, H), F32, kind="ExternalOutput")

    with tile.TileContext(nc) as tc:
        with (
            tc.tile_pool(name="xp", bufs=1) as xp,
            tc.tile_pool(name="wp", bufs=2) as wp,
            tc.tile_pool(name="bp", bufs=2) as bp,
            tc.tile_pool(name="cp", bufs=1) as cp,
            tc.tile_pool(name="gp", bufs=1) as gp,
            tc.tile_pool(name="rp", bufs=3) as rp,
            tc.tile_pool(name="accp", bufs=1) as accp,
            tc.tile_pool(name="ps", bufs=3, space="PSUM") as ps,
            tc.tile_pool(name="psg", bufs=2, space="PSUM") as psg,
        ):
            # ---- constants / activations in SBUF ----
            xT = xp.tile([P, KC, BL], BF16)
            nc.sync.dma_start(xT[:], xT_d.rearrange("(c p) b -> p c b", p=P))
            wg = cp.tile([P, KC, T * NG], BF16, tag="wg")
            nc.sync.dma_start(wg[:], wg_d.rearrange("(c p) g -> p c g", p=P))
            ones = cp.tile([1, P], BF16, tag="ones")
            nc.vector.memset(ones[:], 1.0)

            if bias_mode == "act":
                # One-time: set the has_written bits of every PSUM bank the
                # z-tiles will use, so accumulating (start=False) matmuls on
                # top of the ScalarE bias prefill ADD instead of overwriting
                # on the first use of each bank after a device reset.
                zrow = cp.tile([1, 512], BF16, tag="zrow")
                nc.vector.memset(zrow[:], 0.0)
                for _slot in range(3):
                    zi = ps.tile([P, H], F32, tag="z")
                    for n in range(NH):
                        nc.tensor.matmul(
                            zi[:, n * 512:(n + 1) * 512], ones[:], zrow[:],
                            start=True, stop=True,
                        )

            def emit_body():
                # ---- gates for every batch tile ----
                gates = []
                for i in range(NT):
                    pg = psg.tile([P, T * NG], F32)
                    for c in range(KC):
                        nc.tensor.matmul(
                            pg[:],
                            xT[:, c, i * P:(i + 1) * P],
                            wg[:, c, :],
                            start=(c == 0),
                            stop=(c == KC - 1),
                        )
                    ex = gp.tile([P, T * NG], F32, tag=f"ex{i}")
                    nc.scalar.activation(ex[:], pg[:], ACT.Exp)
                    s = gp.tile([P, T], F32, tag=f"gs{i}")
                    for t in range(T):
                        nc.vector.tensor_reduce(
                            s[:, t:t + 1], ex[:, t * NG:(t + 1) * NG],
                            axis=AXIS.X, op=ALU.add,
                        )
                    rcp = gp.tile([P, T], F32, tag=f"gr{i}")
                    nc.vector.reciprocal(rcp[:], s[:])
                    g = gp.tile([P, T * NG], F32, tag=f"g{i}")
                    for t in range(T):
                        nc.vector.tensor_scalar(
                            g[:, t * NG:(t + 1) * NG],
                            ex[:, t * NG:(t + 1) * NG],
                            rcp[:, t:t + 1], None, op0=ALU.mult,
                        )
                    gates.append(g)

                # ---- expert loop (shared experts first) ----
                acc = {}
                for e in range(NE):
                    w = wp.tile([P, KC, H], BF16)
                    nc.sync.dma_start(
                        w[:], w_d[e].rearrange("(c p) h -> p c h", p=P))
                    if bias_mode == "act":
                        be = bp.tile([P, H], BF16)
                        nc.sync.dma_start(be[:], b_d[e])
                    else:
                        be = bp.tile([1, H], BF16)
                        nc.sync.dma_start(be[:], b_d[e][None, :])

                    for i in range(NT):
                        z = ps.tile([P, H], F32, tag="z")
                        if bias_mode == "act":
                            nc.scalar.copy(z[:], be[:])
                        else:
                            for n in range(NH):
                                nc.tensor.matmul(
                                    z[:, n * 512:(n + 1) * 512],
                                    ones[:],
                                    be[:, n * 512:(n + 1) * 512],
                                    start=True, stop=False,
                                )
                        for c in range(KC):
                            lhsT = xT[:, c, i * P:(i + 1) * P]
                            for n in range(NH):
                                nc.tensor.matmul(
                                    z[:, n * 512:(n + 1) * 512],
                                    lhsT,
                                    w[:, c, n * 512:(n + 1) * 512],
                                    start=False, stop=(c == KC - 1),
                                    skip_group_check=(bias_mode == "act"),
                                )
                        r = rp.tile([P, H], F32)
                        nc.scalar.activation(r[:], z[:], ACT.Relu)
                        for (t, col) in _consumers(e):
                            gcol = gates[i][:, col:col + 1]
                            if (t, i) not in acc:
                                a = accp.tile([P, H], F32, tag=f"acc{t}_{i}")
                                acc[(t, i)] = a
                                nc.vector.tensor_scalar(
                                    a[:], r[:], gcol, None, op0=ALU.mult,
                                )
                            else:
                                a = acc[(t, i)]
                                nc.vector.scalar_tensor_tensor(
                                    a[:], r[:], gcol, a[:],
                                    op0=ALU.mult, op1=ALU.add,
                                )

                    # task t is complete once its specific experts are done
                    if e >= NSHARED and (e - NSHARED) % NSPEC == NSPEC - 1:
                        t_done = (e - NSHARED) // NSPEC
                        for i in range(NT):
                            nc.sync.dma_start(
                                out_d[t_done, i * P:(i + 1) * P, :],
                                acc[(t_done, i)][:],
                            )

            for _ in range(repeat):
                emit_body()

    nc.compile()
    return nc


_NC_CACHE = None


def _get_nc():
    global _NC_CACHE
    if _NC_CACHE is None:
        _NC_CACHE = _build_nc()
    return _NC_CACHE


def prep_inputs(x, Ws, bs, Wt, bt, Wg):
    """Host-side shard/cast/transpose: returns per-core input maps."""
    x = np.asarray(x)
    # expert order: shared(4) then task-specific t-major (12)
    w_all = np.concatenate(
        [np.asarray(Ws), np.asarray(Wt).reshape(T * NSPEC, D, H)], axis=0
    ).astype(BF16_NP)                                  # (16, D, H)
    b_all = np.concatenate(
        [np.asarray(bs), np.asarray(bt).reshape(T * NSPEC, H)], axis=0
    ).astype(BF16_NP)                                  # (16, H)
    if BIAS_MODE == "act":
        b_all = np.ascontiguousarray(
            np.broadcast_to(b_all[:, None, :], (NE, P, H)))  # (16, 128, H)
    # reference gate candidate order is [specific(4), shared(4)]; our
    # gate column layout is t*8 + [0..3]=specific j, [4..7]=shared s.
    wg_all = np.ascontiguousarray(
        np.asarray(Wg).transpose(1, 0, 2).reshape(D, T * NG)
    ).astype(BF16_NP)                                  # (D, 24)

    in_maps = []
    for c in range(N_CORES):
        xs = x[c * BL:(c + 1) * BL]                    # (BL, D)
        xT = np.ascontiguousarray(xs.T).astype(BF16_NP)  # (D, BL)
        in_maps.append({"xT": xT, "W": w_all, "bias": b_all, "Wg": wg_all})
    return in_maps


def kernel(x, Ws, bs, Wt, bt, Wg):
    """Full-input entry point: shard, run on 8 cores, gather."""
    in_maps = prep_inputs(x, Ws, bs, Wt, bt, Wg)
    nc = _get_nc()
    res = run_bass_kernel_spmd(nc, in_maps, core_ids=list(range(N_CORES)))
    out = np.concatenate([res.results[c]["out"] for c in range(N_CORES)], axis=1)
    return out
